# revision 21
# baseline (speedup 1.0000x reference)
"""GAT message-passing kernel for 8 trn2 NeuronCores (v2).

Math (reference):
    Wx = x @ W;  s1 = Wx@a1/sqrt(2D);  s2 = Wx@a2/sqrt(2D)   (per t)
    weight = softmax_m(lrelu(s1[m] + s2[n]));  agg = lrelu(weight @ Wx)
    out = x - agg

Key identities:
  * exp(lrelu(s1+s2)) = max(exp(s1+s2), exp(0.01(s1+s2))); dividing the
    softmax row by exp(s2[n]) (softmax-invariant):
        et_true[m,n] = F1[m] * max(q[m], r[n])
    with q = exp(0.99*s1), F1 = exp(0.01*s1), r = exp(-0.99*s2).
  * F1 = exp(0.01*s1) in [0.995, 1.005] is dropped (F1 ~= 1): verified
    1.3e-4 RMS contribution vs the 2e-2 gate. The score tile is then ONE
    single-op DVE tensor_scalar max(r_b, q[m]) which runs in 4x mode
    (the 2-op mult+max form caps at 2x).
  * The moving operand is [Wx | -1] so the accumulator's col 128 is -Z
    (exact softmax denominator): rz = reciprocal(-Z) = -1/Z and, since
    lrelu is positively homogeneous, lrelu(num/Z) = lrelu(num)/Z, the
    finalize is lrelu(whole PSUM trio) then fused (lr * rz) + xn.

Sharding: 8 cores = 4 t-slices x 2 N-halves; each core aggregates over all
4096 source nodes for its own (t, 2048 dest nodes).

Loop: mt-outer, software-pipelined: proj(mt+2) and et(mt+1) are emitted
ahead of mt's 16 accumulation matmuls (4 q x 4 j chunks) so the
proj -> q(ACT exp) -> et(DVE max) chain hides under the PE groups.
PSUM: 2 banks scratch (proj/r_b, shared tag) + 6 banks for the 16
accumulators packed three-per-bank.
"""

import sys

if "/opt/trn_rl_repo" not in sys.path:
    sys.path.insert(0, "/opt/trn_rl_repo")

import numpy as np

N, T, D = 4096, 4, 128
P = 128
HALF = N // 2            # 2048 dest nodes per core
MT = N // P              # 32 source tiles
NQ = HALF // 512         # 4 dest chunks of 512
DC = D + 1               # moving operand cols: [F1*Wx | F1]
SCALE_INV = 1.0 / 16.0   # 1/sqrt(2*128)

_CACHE = {}


def _build():
    import concourse.mybir as mybir
    from concourse import bacc
    from concourse.tile import TileContext

    f32 = mybir.dt.float32
    bf16 = mybir.dt.bfloat16
    Alu = mybir.AluOpType
    Act = mybir.ActivationFunctionType

    nc = bacc.Bacc()
    # wp packs [W | w1 | w2-as-2-bf16-bitcast]: one DMA for all params
    wp = nc.declare_dram_parameter("wp", [P, DC + 3], bf16, isOutput=False)
    xt = nc.declare_dram_parameter("xt", [P, N], bf16, isOutput=False)
    xn = nc.declare_dram_parameter("xn", [P, HALF], f32, isOutput=False)
    out = nc.declare_dram_parameter("out", [HALF, D], f32, isOutput=True)

    with TileContext(nc) as tc:
        with (
            tc.tile_pool(name="const", bufs=1) as cpool,
            tc.tile_pool(name="epool", bufs=3) as epool,
            tc.tile_pool(name="fpool", bufs=4) as fpool,
            tc.tile_pool(name="opool", bufs=2) as opool,
        ):
            # ---- input DMAs: params first, then xT halves, xn last ----
            wpx_sb = cpool.tile([P, DC + 3], bf16)
            nc.sync.dma_start(wpx_sb[:, :], wp[:, :])
            wp_sb = wpx_sb[:, 0:DC]
            w2_sb = wpx_sb.bitcast(f32)[:, (DC + 1) // 2 : (DC + 1) // 2 + 1]
            xt_h = [
                cpool.tile([P, 1024], bf16, name=f"xt{ch}") for ch in range(4)
            ]
            for ch in range(4):
                nc.sync.dma_start(
                    xt_h[ch][:, :], xt[:, ch * 1024 : (ch + 1) * 1024]
                )
            # xn rides the sync queue after xt: a gpsimd-queue DMA would
            # make the entry barrier drain wait for the full transfer.
            xn_sb = cpool.tile([P, HALF], f32)
            nc.sync.dma_start(xn_sb[:, :], xn[:, :])

            # persistent SBUF state
            wxm = cpool.tile([P, MT * DC], bf16)     # [Wx | -1] per mt
            r_b = cpool.tile([P, HALF], bf16)        # exp(-0.99 s2[n]) rows
            q_all = cpool.tile([P, MT], f32)         # exp(0.99 s1)

            with (
                tc.tile_pool(name="spsum", bufs=2, space="PSUM") as spool,
                tc.tile_pool(name="mpsum", bufs=1, space="PSUM") as mpool,
            ):
                # ---- w2b[k,m] = w2[k] broadcast: rank-1 stationary ----
                w2b = cpool.tile([P, P], bf16)
                nc.vector.tensor_scalar(
                    w2b[:, :], wp_sb[:, 0:P], 0.0, w2_sb[:, :],
                    Alu.mult, Alu.add,
                )
                # constant -1 column in every moving-operand slot: the
                # accumulated col 128 is then -Z, so reciprocal gives -1/Z
                # and the finalize is a fused (lr * rz) + xn.
                nc.scalar.activation(
                    wxm.rearrange("p (m c) -> p m c", c=DC)[:, :, D : D + 1],
                    wp_sb[:, 0:MT],
                    Act.Copy, scale=0.0, bias=-1.0,
                )
                # ---- PE warm-up: throwaway matmuls fill the whole
                # DMA-wait window (no input dependency via memset source)
                # so the HAM clock gate reaches full rate before the real
                # matmuls start; idle re-throttles it within ~1us.
                wsrc = cpool.tile([P, P], f32)
                nc.gpsimd.memset(wsrc, 0.0)
                warm = spool.tile([P, 512], f32, tag="sc", name="warm")
                for _ in range(7):
                    nc.tensor.matmul(
                        warm[:, 0:P], wsrc[:, :], wsrc[:, :],
                        start=True, stop=True,
                    )

                # ---- r_b = exp(-0.99 * s2) via replicated-row matmuls ----
                def emit_rb(c):
                    rb_ps = spool.tile([P, 512], f32, tag="sc", name="rb_ps")
                    nc.tensor.matmul(
                        rb_ps[:, :], w2b[:, :],
                        xt_h[c // 2][:, (c % 2) * 512 : (c % 2 + 1) * 512],
                        start=True, stop=True,
                    )
                    nc.scalar.activation(
                        r_b[:, c * 512 : (c + 1) * 512], rb_ps[:, :],
                        Act.Exp, scale=-0.99,
                    )

                # ---- 16 accumulators packed 3-per-bank: 5 trios + 1 ----
                trio = [
                    mpool.tile([P, 3 * DC], f32, tag=f"tr{t}", name=f"tr{t}")
                    for t in range(5)
                ] + [mpool.tile([P, DC], f32, tag="tr5", name="tr5")]

                def acc_view(q, j):
                    idx = q * 4 + j
                    t, s = idx // 3, idx % 3
                    return trio[t][:, s * DC : (s + 1) * DC]

                # ---- main loop: mt-outer, software-pipelined ----
                # proj(mt) is emitted two groups ahead of its matmuls so the
                # proj -> q(ACT) -> et(DVE) chain hides under group mt-2/-1.
                def emit_proj(mt):
                    p_ps = spool.tile([P, 512], f32, tag="sc", name="p_ps")
                    nc.tensor.matmul(
                        p_ps[:, 0:DC],
                        xt_h[mt // 8][:, (mt % 8) * P : (mt % 8 + 1) * P],
                        wp_sb[:, :],
                        start=True, stop=True,
                    )
                    nc.scalar.activation(
                        q_all[:, mt : mt + 1], p_ps[:, D : D + 1],
                        Act.Exp, scale=0.99,
                    )
                    nc.scalar.activation(
                        wxm[:, mt * DC : mt * DC + D], p_ps[:, 0:D], Act.Copy
                    )

                def emit_et(mt, h):
                    # half h covers dest q-chunks 2h, 2h+1: only needs the
                    # matching r_b half, so group 0 starts before all of
                    # r_b is ready.
                    et = epool.tile([P, 1024], bf16, name=f"et{h}", tag=f"et{h}")
                    nc.vector.tensor_scalar(
                        et[:, :], r_b[:, h * 1024 : (h + 1) * 1024],
                        q_all[:, mt : mt + 1], None, Alu.max,
                    )
                    return et

                emit_rb(0)
                emit_rb(1)
                emit_proj(0)
                emit_proj(1)
                emit_rb(2)
                emit_rb(3)
                ets = {(0, 0): emit_et(0, 0), (0, 1): emit_et(0, 1)}
                for mt in range(MT):
                    if mt + 2 < MT:
                        emit_proj(mt + 2)
                    if mt + 1 < MT:
                        ets[(mt + 1, 0)] = emit_et(mt + 1, 0)
                        ets[(mt + 1, 1)] = emit_et(mt + 1, 1)
                    wv = wxm[:, mt * DC : (mt + 1) * DC]
                    for q in range(NQ):
                        et = ets[(mt, q // 2)]
                        off = (q % 2) * 512
                        for j in range(4):
                            nc.tensor.matmul(
                                acc_view(q, j),
                                et[:, off + j * P : off + (j + 1) * P],
                                wv,
                                start=(mt == 0),
                                stop=(mt == MT - 1),
                            )
                    del ets[(mt, 0)], ets[(mt, 1)]

                # ---- finalize: lrelu whole trios, then (lr*(-1/Z)) + xn ----
                lrt = [
                    fpool.tile([P, 3 * DC], f32, tag=f"lrt{t}", name=f"lrt{t}")
                    for t in range(5)
                ] + [fpool.tile([P, DC], f32, tag="lrt5", name="lrt5")]
                for t in range(6):
                    nc.scalar.activation(
                        lrt[t][:, :], trio[t][:, :], Act.Lrelu, alpha=0.01
                    )

                def lr_view(q, j):
                    idx = q * 4 + j
                    t, s = idx // 3, idx % 3
                    return lrt[t][:, s * DC : (s + 1) * DC]

                rzs = fpool.tile([P, 16], f32, tag="rzs", name="rzs")
                for q in range(NQ):
                    for j in range(4):
                        nc.vector.reciprocal(
                            rzs[:, q * 4 + j : q * 4 + j + 1],
                            acc_view(q, j)[:, D : D + 1],
                        )
                for q in range(NQ):
                    o_q = opool.tile([P, 512], f32, name="o_q")
                    for j in range(4):
                        nc.vector.scalar_tensor_tensor(
                            o_q[:, j * P : (j + 1) * P],
                            lr_view(q, j)[:, :D],
                            rzs[:, q * 4 + j : q * 4 + j + 1],
                            xn_sb[:, q * 512 + j * P : q * 512 + (j + 1) * P],
                            Alu.mult,
                            Alu.add,
                        )
                    out_view = out[q * 512 : (q + 1) * 512, :].rearrange(
                        "(j p) d -> p j d", p=P
                    )
                    nc.sync.dma_start(
                        out_view, o_q.rearrange("p (j d) -> p j d", j=4)
                    )

    nc.compile()
    return nc


def _prep_inputs(x, W, a1, a2):
    """Per-core packed inputs. Core c: t = c//2, n-half h = c%2.

    xT is host-rotated so the core's own 2048 dest columns come first
    (a rotation does not change a sum over all source nodes).
    """
    import ml_dtypes

    x = np.asarray(x, dtype=np.float32)
    W = np.asarray(W, dtype=np.float32)
    w1 = (W @ np.asarray(a1, np.float32)) * SCALE_INV
    w2 = (W @ np.asarray(a2, np.float32)) * SCALE_INV
    wp_bf = np.concatenate([W, w1[:, None]], axis=1).astype(ml_dtypes.bfloat16)
    w2_pair = np.ascontiguousarray(w2[:, None].astype(np.float32)).view(
        ml_dtypes.bfloat16
    )
    pad = np.zeros((P, 2), dtype=ml_dtypes.bfloat16)
    wpx = np.ascontiguousarray(
        np.concatenate([wp_bf, pad[:, :1], w2_pair], axis=1)
    )
    in_maps = []
    for c in range(8):
        t, h = c // 2, c % 2
        xt = x[:, t, :].T  # [D, N]
        if h == 1:
            xt = np.concatenate([xt[:, HALF:], xt[:, :HALF]], axis=1)
        xn = x[h * HALF : (h + 1) * HALF, t, :]  # [2048, 128]
        xn_packed = (
            xn.reshape(HALF // P, P, D).transpose(1, 0, 2).reshape(P, HALF)
        )
        in_maps.append(
            {
                "wp": wpx,
                "xt": np.ascontiguousarray(xt.astype(ml_dtypes.bfloat16)),
                "xn": np.ascontiguousarray(xn_packed),
            }
        )
    return in_maps


def _run(x, W, a1, a2, trace=False):
    from concourse.bass_utils import run_bass_kernel_spmd

    key = "nc"
    if key not in _CACHE:
        _CACHE[key] = _build()
    nc = _CACHE[key]
    in_maps = _prep_inputs(x, W, a1, a2)
    res = run_bass_kernel_spmd(nc, in_maps, list(range(8)), trace=trace)
    out_full = np.empty((N, T, D), dtype=np.float32)
    for c in range(8):
        t, h = c // 2, c % 2
        out_full[h * HALF : (h + 1) * HALF, t, :] = res.results[c]["out"]
    return out_full, res


def kernel(x, W, a1, a2):
    out, _ = _run(x, W, a1, a2, trace=False)
    return out
